# revision 7
# baseline (speedup 1.0000x reference)
"""BiLSTM-CRF Trainium2 kernel (8-core data-parallel over batch).

Contract: kernel(**inputs) takes the full unsharded inputs of the reference
(sentence, emb, LSTM weights, W_out/b_out, transitions) and returns
(score [B] f32, path [B, L] int32), computed on 8 NeuronCores via
bass/Tile with run_bass_kernel_spmd.

Per-core work (4 sequences):
  1. indirect-DMA gather of embedding rows (bf16 table), PE-transpose to x^T
  2. batched xW = W_ih @ x^T (+bias via K=1 ones matmul) for both directions
  3. single fused LSTM pass: fwd chain (t=0..511) and bwd chain (t=511..0)
     interleaved; bf16 stationary weights, gates in PSUM, Sigmoid/Tanh on
     ScalarE, cell update on VectorE/GpSimd; h trajectories stored bf16
  4. batched feats^T = W_out @ [h_f; h_b] (+b_out)
  5. Viterbi as two interleaved max-plus scans (forward fv and backward bv)
     in f32; per-step state transpose/replicate via tiny fp32 matmuls
  6. path[t] = argmax_p(fv_t + bv_t - feat_t) batched; score from terminal
"""

import os
import numpy as np
import ml_dtypes

import concourse.bass as bass
import concourse.bacc as bacc
import concourse.mybir as mybir
import concourse.tile as tile
from concourse.bass_utils import run_bass_kernel_spmd

f32 = mybir.dt.float32
bf16 = mybir.dt.bfloat16
i32 = mybir.dt.int32
AF = mybir.ActivationFunctionType
Alu = mybir.AluOpType

N_CORES = 8
B_GLOB, L, E, HID = 32, 512, 256, 512
H = HID // 2
T, START, STOP = 12, 10, 11
NEG = -10000.0
BL = B_GLOB // N_CORES  # 4 seqs per core
NTOK = BL * L           # 2048 tokens per core
NJ = NTOK // 128        # 16 gather columns


def _bf(x):
    return np.asarray(x, dtype=np.float32).astype(ml_dtypes.bfloat16)


def build_kernel():
    nc = bacc.Bacc("TRN2", target_bir_lowering=False, debug=False)

    emb_d = nc.dram_tensor("embt", [50000, E], bf16, kind="ExternalInput")
    idx_d = nc.dram_tensor("idx", [128, NJ], i32, kind="ExternalInput")
    wl_d = nc.dram_tensor("wl", [2, 16, 128, 128], bf16, kind="ExternalInput")
    wi_d = nc.dram_tensor("wi", [2, 16, 128, 128], bf16, kind="ExternalInput")
    bl_d = nc.dram_tensor("bl", [2, 8, 128], bf16, kind="ExternalInput")
    wo_d = nc.dram_tensor("wo", [4, 128, T], bf16, kind="ExternalInput")
    bo_d = nc.dram_tensor("bo", [1, T], bf16, kind="ExternalInput")
    id128_d = nc.dram_tensor("id128", [128, 128], bf16, kind="ExternalInput")
    i12_d = nc.dram_tensor("i12", [T, T], f32, kind="ExternalInput")
    t4f_d = nc.dram_tensor("t4f", [T, BL * T], f32, kind="ExternalInput")
    t4b_d = nc.dram_tensor("t4b", [T, BL * T], f32, kind="ExternalInput")
    fv0_d = nc.dram_tensor("fv0", [1, BL * T], f32, kind="ExternalInput")
    tstop_d = nc.dram_tensor("tstop", [T, BL], f32, kind="ExternalInput")
    iota_d = nc.dram_tensor("iota", [128, T], f32, kind="ExternalInput")

    score_d = nc.dram_tensor("score", [1, BL], f32, kind="ExternalOutput")
    path_d = nc.dram_tensor("path", [128, NJ], i32, kind="ExternalOutput")

    with tile.TileContext(nc) as tc:
        with (
            tc.tile_pool(name="consts", bufs=1) as consts,
            tc.tile_pool(name="big", bufs=1) as big,
            tc.tile_pool(name="work", bufs=4) as work,
        ):
            # ---------- load constants ----------
            wl_sb = consts.tile([128, 2, 16, 128], bf16, tag="wl")
            nc.sync.dma_start(out=wl_sb, in_=wl_d[:, :, :, :].rearrange("d n p q -> p d n q"))
            wi_sb = consts.tile([128, 2, 16, 128], bf16, tag="wi")
            nc.sync.dma_start(out=wi_sb, in_=wi_d[:, :, :, :].rearrange("d n p q -> p d n q"))
            bl_sb = consts.tile([1, 2, 8, 128], bf16, tag="bl")
            nc.sync.dma_start(out=bl_sb, in_=bl_d[:, :, :].rearrange("d m q -> (d m q)").unsqueeze(0))
            wo_sb = consts.tile([128, 4, T], bf16, tag="wo")
            nc.sync.dma_start(out=wo_sb, in_=wo_d[:, :, :].rearrange("k p t -> p k t"))
            bo_sb = consts.tile([1, T], bf16, tag="bo")
            nc.sync.dma_start(out=bo_sb, in_=bo_d[:, :])
            id128 = consts.tile([128, 128], bf16, tag="id128")
            nc.sync.dma_start(out=id128, in_=id128_d[:, :])
            i12_sb = consts.tile([T, T], f32, tag="i12")
            nc.sync.dma_start(out=i12_sb, in_=i12_d[:, :])
            t4f_sb = consts.tile([T, BL * T], f32, tag="t4f")
            nc.sync.dma_start(out=t4f_sb, in_=t4f_d[:, :])
            t4b_sb = consts.tile([T, BL * T], f32, tag="t4b")
            nc.sync.dma_start(out=t4b_sb, in_=t4b_d[:, :])
            fv0_sb = consts.tile([1, BL * T], f32, tag="fv0")
            nc.sync.dma_start(out=fv0_sb, in_=fv0_d[:, :])
            tstop_sb = consts.tile([T, BL], f32, tag="tstop")
            nc.sync.dma_start(out=tstop_sb, in_=tstop_d[:, :])
            ones512 = consts.tile([1, 512], bf16, tag="ones512")
            nc.vector.memset(ones512, 1.0)
            iota_sb = consts.tile([128, T], f32, tag="iota")
            nc.sync.dma_start(out=iota_sb, in_=iota_d[:, :])
            ones1f = consts.tile([1, T], f32, tag="ones1f")
            nc.vector.memset(ones1f, 1.0)
            idx_sb = consts.tile([128, NJ], i32, tag="idx")
            nc.sync.dma_start(out=idx_sb, in_=idx_d[:, :])

            # ---------- 1) embedding gather (token u = 4t+s at [u%128, u//128]) ----------
            sc_gather = nc.enter_named_scope("ph_gather", False)
            x_tok = big.tile([128, NJ, E], bf16, tag="xtok")
            for j in range(NJ):
                nc.gpsimd.indirect_dma_start(
                    out=x_tok[:, j, :], out_offset=None,
                    in_=emb_d[:, :],
                    in_offset=bass.IndirectOffsetOnAxis(ap=idx_sb[:, j:j + 1], axis=0),
                )

            # ---------- transpose to x^T [128, 2(kc), NTOK] ----------
            xT = big.tile([128, 2, NTOK], bf16, tag="xT")
            with tc.tile_pool(name="ps_tr", bufs=4, space="PSUM") as ps_tr:
                for j in range(NJ):
                    for kc in range(2):
                        tp = ps_tr.tile([128, 128], f32, tag="tp")
                        nc.tensor.matmul(
                            tp, x_tok[:, j, kc * 128:(kc + 1) * 128], id128,
                            start=True, stop=True)
                        nc.vector.tensor_copy(xT[:, kc, j * 128:(j + 1) * 128], tp)

            nc.leave_named_scope("ph_gather", sc_gather[0], False)
            sc_xw = nc.enter_named_scope("ph_xw", False)
            # ---------- 2) xW precompute ----------
            # xw_buf[d] [128, 8(mc), L, BL] bf16 ; tokens are t-major so an
            # N=512 chunk c covers tokens [512c, 512c+512) = t in [128c,128c+128) x 4 seqs
            xw_buf = big.tile([128, 2, 8, L * BL], bf16, tag="xw")
            with tc.tile_pool(name="ps_xw", bufs=4, space="PSUM") as ps_xw:
                for d in range(2):
                    for mc in range(8):
                        for c in range(NTOK // 512):
                            pxw = ps_xw.tile([128, 512], f32, tag="pxw")
                            for kc in range(2):
                                nc.tensor.matmul(
                                    pxw, wi_sb[:, d, mc * 2 + kc, :],
                                    xT[:, kc, c * 512:(c + 1) * 512],
                                    start=(kc == 0), stop=False)
                            nc.tensor.matmul(
                                pxw, bl_sb[:, d, mc, :],
                                ones512,
                                start=False, stop=True)
                            nc.vector.tensor_copy(
                                xw_buf[:, d, mc, c * 512:(c + 1) * 512], pxw)

            nc.leave_named_scope("ph_xw", sc_xw[0], False)
            sc_lstm = nc.enter_named_scope("ph_lstm", False)
            # ---------- 3) LSTM fused pass ----------
            hT_buf = big.tile([128, 2, 2, L, BL], bf16, tag="hT")  # [p, d, kc, t, s]
            h_init = consts.tile([128, 2, BL], bf16, tag="hinit")
            nc.vector.memset(h_init.rearrange("p k b -> p (k b)"), 0.0)
            c_init = []
            for d in range(2):
                ci = consts.tile([128, 2 * BL], f32, tag=f"cinit{d}")
                nc.vector.memset(ci, 0.0)
                c_init.append(ci)
            c_cur = list(c_init)

            with (
                tc.tile_pool(name="ps_g", bufs=2, space="PSUM") as ps_g,
                tc.tile_pool(name="lwork", bufs=4) as lwork,
                tc.tile_pool(name="lstate", bufs=3) as lstate,
            ):
                for step in range(L):
                    for d in range(2):
                        tp = step if d == 0 else L - 1 - step
                        tprev = tp - 1 if d == 0 else tp + 1
                        if step == 0:
                            h_prev = h_init[:, :, :]
                        else:
                            h_prev = hT_buf[:, d, :, tprev, :]
                        c_prev = c_cur[d]
                        g_ps = ps_g.tile([128, 8, BL], f32, tag=f"g{d}")
                        for mc in range(8):
                            for kc in range(2):
                                nc.tensor.matmul(
                                    g_ps[:, mc, :],
                                    wl_sb[:, d, mc * 2 + kc, :],
                                    h_prev[:, kc, :],
                                    start=(kc == 0), stop=(kc == 1))
                        g_sb = lwork.tile([128, 8 * BL], f32, tag=f"gsb{d}")
                        nc.vector.tensor_add(
                            g_sb.rearrange("p (m b) -> p m b", m=8),
                            g_ps,
                            xw_buf[:, d, :, tp * BL:(tp + 1) * BL])
                        sg = lwork.tile([128, 8 * BL], f32, tag=f"sg{d}")
                        nc.scalar.activation(sg, g_sb, AF.Sigmoid)
                        tg = lwork.tile([128, 2 * BL], f32, tag=f"tg{d}")
                        nc.scalar.activation(tg, g_sb[:, 4 * BL:6 * BL], AF.Tanh)
                        p_t = lwork.tile([128, 2 * BL], f32, tag=f"p{d}")
                        nc.gpsimd.tensor_mul(p_t, sg[:, 0:2 * BL], tg)
                        m_t = lwork.tile([128, 2 * BL], f32, tag=f"m{d}")
                        nc.vector.tensor_mul(m_t, sg[:, 2 * BL:4 * BL], c_prev)
                        c2 = lstate.tile([128, 2 * BL], f32, tag=f"c{d}")
                        nc.vector.tensor_add(c2, p_t, m_t)
                        th = lwork.tile([128, 2 * BL], f32, tag=f"th{d}")
                        nc.scalar.activation(th, c2, AF.Tanh)
                        nc.gpsimd.tensor_mul(
                            hT_buf[:, d, :, tp, :],
                            sg[:, 6 * BL:8 * BL].rearrange("p (k b) -> p k b", k=2),
                            th.rearrange("p (k b) -> p k b", k=2))
                        c_cur[d] = c2

            nc.leave_named_scope("ph_lstm", sc_lstm[0], False)
            sc_feat = nc.enter_named_scope("ph_feats", False)
            # ---------- 4) feats^T [T, L, BL] ----------
            featsT = big.tile([T, L, BL], f32, tag="featsT")
            with tc.tile_pool(name="ps_f", bufs=4, space="PSUM") as ps_f:
                for c in range(NTOK // 512):
                    pf = ps_f.tile([T, 512], f32, tag="pf")
                    for d in range(2):
                        for kc in range(2):
                            nc.tensor.matmul(
                                pf, wo_sb[:, d * 2 + kc, :],
                                hT_buf[:, d, kc, c * 128:(c + 1) * 128, :].rearrange("p t b -> p (t b)"),
                                start=(d == 0 and kc == 0), stop=False)
                    nc.tensor.matmul(
                        pf, bo_sb[:, :], ones512,
                        start=False, stop=True)
                    nc.vector.tensor_copy(
                        featsT[:, c * 128:(c + 1) * 128, :].rearrange("p t b -> p (t b)"), pf)

            nc.leave_named_scope("ph_feats", sc_feat[0], False)
            sc_vit = nc.enter_named_scope("ph_viterbi", False)
            # ---------- 5) Viterbi double scan ----------
            fv_buf = big.tile([T, L, BL], f32, tag="fv")
            bv_buf = big.tile([T, L, BL], f32, tag="bv")

            with (
                tc.tile_pool(name="ps_v", bufs=4, space="PSUM") as ps_v,
                tc.tile_pool(name="vwork", bufs=4) as vwork,
            ):
                # fwd init: replicate fv0 flat -> psum [T, BL*T]
                repf = ps_v.tile([T, BL * T], f32, tag="rep")
                nc.tensor.matmul(repf, ones1f, fv0_sb,
                                 start=True, stop=True)
                # bwd init: bv[511] = tstop + feat[511]
                nc.vector.tensor_add(bv_buf[:, L - 1, :], tstop_sb, featsT[:, L - 1, :])
                trb = ps_v.tile([1, BL * T], f32, tag="tr")
                for s in range(BL):
                    nc.tensor.matmul(trb[:, s * T:(s + 1) * T],
                                     bv_buf[:, L - 1, s:s + 1], i12_sb,
                                     start=True, stop=True)
                bvT = vwork.tile([1, BL * T], f32, tag="bvT")
                nc.vector.tensor_copy(bvT, trb)
                repb = ps_v.tile([T, BL * T], f32, tag="rep")
                nc.tensor.matmul(repb, ones1f, bvT,
                                 start=True, stop=True)

                for step in range(L):
                    # forward chain at t = step
                    t_f = step
                    scf = vwork.tile([T, BL * T], f32, tag="scf")
                    nc.vector.tensor_add(scf, t4f_sb, repf)
                    mxf = vwork.tile([T, BL], f32, tag="mxf")
                    nc.vector.tensor_reduce(
                        mxf.unsqueeze(-1), scf.rearrange("p (s j) -> p s j", s=BL),
                        axis=mybir.AxisListType.X, op=Alu.max)
                    nc.vector.tensor_add(fv_buf[:, t_f, :], mxf, featsT[:, t_f, :])
                    if step < L - 1:
                        trf = ps_v.tile([1, BL * T], f32, tag="tr")
                        for s in range(BL):
                            nc.tensor.matmul(trf[:, s * T:(s + 1) * T],
                                             fv_buf[:, t_f, s:s + 1], i12_sb,
                                             start=True, stop=True)
                        fvT = vwork.tile([1, BL * T], f32, tag="fvT")
                        nc.vector.tensor_copy(fvT, trf)
                        repf = ps_v.tile([T, BL * T], f32, tag="rep")
                        nc.tensor.matmul(repf, ones1f, fvT,
                                         start=True, stop=True)
                    # backward chain at t = L-2-step
                    t_b = L - 2 - step
                    if t_b >= 0:
                        scb = vwork.tile([T, BL * T], f32, tag="scb")
                        nc.vector.tensor_add(scb, t4b_sb, repb)
                        mxb = vwork.tile([T, BL], f32, tag="mxb")
                        nc.vector.tensor_reduce(
                            mxb.unsqueeze(-1), scb.rearrange("p (s j) -> p s j", s=BL),
                            axis=mybir.AxisListType.X, op=Alu.max)
                        nc.vector.tensor_add(bv_buf[:, t_b, :], mxb, featsT[:, t_b, :])
                        if t_b > 0:
                            trb = ps_v.tile([1, BL * T], f32, tag="tr")
                            for s in range(BL):
                                nc.tensor.matmul(trb[:, s * T:(s + 1) * T],
                                                 bv_buf[:, t_b, s:s + 1], i12_sb,
                                                 start=True, stop=True)
                            bvT = vwork.tile([1, BL * T], f32, tag="bvT")
                            nc.vector.tensor_copy(bvT, trb)
                            repb = ps_v.tile([T, BL * T], f32, tag="rep")
                            nc.tensor.matmul(repb, ones1f, bvT,
                                             start=True, stop=True)

                nc.leave_named_scope("ph_viterbi", sc_vit[0], False)
                sc_path = nc.enter_named_scope("ph_path", False)
                # ---------- 6) score + path ----------
                term = vwork.tile([T, BL], f32, tag="term")
                nc.vector.tensor_add(term, fv_buf[:, L - 1, :], tstop_sb)
                trs = ps_v.tile([1, BL * T], f32, tag="tr")
                for s in range(BL):
                    nc.tensor.matmul(trs[:, s * T:(s + 1) * T],
                                     term[:, s:s + 1], i12_sb, start=True, stop=True)
                termT = vwork.tile([1, BL * T], f32, tag="termT")
                nc.vector.tensor_copy(termT, trs)
                score_sb = vwork.tile([1, BL], f32, tag="score")
                nc.vector.tensor_reduce(
                    score_sb.unsqueeze(-1),
                    termT.rearrange("p (s j) -> p s j", s=BL),
                    axis=mybir.AxisListType.X, op=Alu.max)
                nc.sync.dma_start(out=score_d[:, :], in_=score_sb)

                # tot = fv + bv - feats  [T, L*BL]
                tot0 = big.tile([T, L, BL], f32, tag="tot0")
                nc.vector.tensor_add(
                    tot0.rearrange("p t b -> p (t b)"),
                    fv_buf.rearrange("p t b -> p (t b)"),
                    bv_buf.rearrange("p t b -> p (t b)"))
                tot = big.tile([T, L, BL], f32, tag="tot")
                nc.vector.tensor_sub(
                    tot.rearrange("p t b -> p (t b)"),
                    tot0.rearrange("p t b -> p (t b)"),
                    featsT.rearrange("p t b -> p (t b)"))
                # transpose tot -> totT [128(u=t*4+s), NJ, T] via PE f32 transposes
                totT = big.tile([128, NJ, T], f32, tag="totT")
                for j in range(NJ):
                    ptt = ps_v.tile([128, T], f32, tag="tr")
                    nc.tensor.matmul(
                        ptt,
                        tot.rearrange("p t b -> p (t b)")[:, j * 128:(j + 1) * 128],
                        i12_sb, start=True, stop=True)
                    nc.vector.tensor_copy(totT[:, j, :], ptt)
                totmax = vwork.tile([128, NJ], f32, tag="totmax")
                nc.vector.tensor_reduce(
                    totmax.unsqueeze(-1), totT,
                    axis=mybir.AxisListType.X, op=Alu.max)
                ohp = big.tile([128, NJ, T], f32, tag="ohp")
                nc.vector.tensor_tensor(
                    out=ohp, in0=totT,
                    in1=totmax.unsqueeze(-1).broadcast_to([128, NJ, T]),
                    op=Alu.is_equal)
                pathv = vwork.tile([128, NJ], f32, tag="pathv")
                wsum = big.tile([128, NJ, T], f32, tag="wsum")
                nc.vector.tensor_mul(
                    wsum, ohp,
                    iota_sb.unsqueeze(1).broadcast_to([128, NJ, T]))
                nc.vector.tensor_reduce(
                    pathv.unsqueeze(-1), wsum,
                    axis=mybir.AxisListType.X, op=Alu.add)
                path_i = vwork.tile([128, NJ], i32, tag="pathi")
                nc.vector.tensor_copy(path_i, pathv)
                nc.sync.dma_start(out=path_d[:, :], in_=path_i)
                nc.leave_named_scope("ph_path", sc_path[0], False)

    nc.compile()
    return nc


def _host_prep(sentence, emb, W_ih_f, W_hh_f, b_f, W_ih_b, W_hh_b, b_b,
               W_out, b_out, transitions):
    """Build per-core input maps."""
    sent = np.asarray(sentence)
    embt = _bf(emb)
    trans = np.asarray(transitions, np.float32)

    def pack_w(W):  # W [4H=1024, 256] -> [16, 128, 128] tiles lhsT[kc-chunk, mc-chunk]
        Wt = np.asarray(W, np.float32).T  # [256, 1024]
        tiles = np.zeros((16, 128, 128), np.float32)
        for mc in range(8):
            for kc in range(2):
                tiles[mc * 2 + kc] = Wt[kc * 128:(kc + 1) * 128, mc * 128:(mc + 1) * 128]
        return _bf(tiles)

    wl = np.stack([pack_w(W_hh_f), pack_w(W_hh_b)])       # [2,16,128,128]
    wi = np.stack([pack_w(W_ih_f), pack_w(W_ih_b)])
    bl = np.stack([_bf(np.asarray(b_f, np.float32).reshape(8, 128)),
                   _bf(np.asarray(b_b, np.float32).reshape(8, 128))])
    WoT = np.asarray(W_out, np.float32).T                 # [512, 12]
    wo = _bf(np.stack([WoT[0:128], WoT[128:256], WoT[256:384], WoT[384:512]]))
    bo = _bf(np.asarray(b_out, np.float32).reshape(1, T))
    id128 = _bf(np.eye(128, dtype=np.float32))
    i12 = np.eye(T, dtype=np.float32)
    t4f = np.tile(trans[:, None, :], (1, BL, 1)).reshape(T, BL * T)
    t4b = np.tile(trans.T[:, None, :], (1, BL, 1)).reshape(T, BL * T)
    fv0 = np.full((BL, T), NEG, np.float32)
    fv0[:, START] = 0.0
    fv0 = fv0.reshape(1, BL * T)
    tstop = np.tile(trans[STOP][:, None], (1, BL)).astype(np.float32)
    iota = np.tile(np.arange(T, dtype=np.float32)[None, :], (128, 1))

    in_maps = []
    for c in range(N_CORES):
        seqs = sent[c * BL:(c + 1) * BL]  # [4, 512]
        idx = np.zeros((128, NJ), np.int32)
        for u in range(NTOK):
            t, s = divmod(u, BL)
            idx[u % 128, u // 128] = seqs[s, t]
        in_maps.append(dict(
            embt=embt, idx=idx, wl=wl, wi=wi, bl=bl, wo=wo, bo=bo,
            id128=id128, i12=i12, t4f=t4f, t4b=t4b, fv0=fv0, tstop=tstop,
            iota=iota,
        ))
    return in_maps


def kernel(sentence, emb, W_ih_f, W_hh_f, b_f, W_ih_b, W_hh_b, b_b,
           W_out, b_out, transitions):
    in_maps = _host_prep(sentence, emb, W_ih_f, W_hh_f, b_f,
                         W_ih_b, W_hh_b, b_b, W_out, b_out, transitions)
    nc = build_kernel()
    trace = os.environ.get("BILSTM_TRACE", "0") == "1"
    res = run_bass_kernel_spmd(nc, in_maps, core_ids=list(range(N_CORES)),
                               trace=trace)
    if trace:
        print(f"HW exec time: {res.exec_time_ns} ns")
        for k, v in sorted((res.per_core_scope_times or {}).items()):
            print(f"  scope {k}: {list(v.values())[0]} ns")
    score = np.zeros((B_GLOB,), np.float32)
    path = np.zeros((B_GLOB, L), np.int32)
    for c in range(N_CORES):
        r = res.results[c]
        score[c * BL:(c + 1) * BL] = r["score"][0]
        pt = r["path"]  # [128, NJ] token-major u = 4t+s
        for u in range(NTOK):
            t, s = divmod(u, BL)
            path[c * BL + s, t] = pt[u % 128, u // 128]
    return score, path


# revision 9
# speedup vs baseline: 1.2207x; 1.2207x over previous
"""BiLSTM-CRF Trainium2 kernel (8-core data-parallel over batch).

Contract: kernel(**inputs) takes the full unsharded inputs of the reference
(sentence, emb, LSTM weights, W_out/b_out, transitions) and returns
(score [B] f32, path [B, L] int32), computed on 8 NeuronCores via
bass/Tile with run_bass_kernel_spmd.

Per-core work (4 sequences):
  1. indirect-DMA gather of embedding rows (bf16 table), PE-transpose to x^T
  2. batched xW = W_ih @ x^T (+bias via K=1 ones matmul) for both directions
  3. single fused LSTM pass: fwd chain (t=0..511) and bwd chain (t=511..0)
     interleaved; bf16 stationary weights, gates in PSUM, Sigmoid/Tanh on
     ScalarE, cell update on VectorE/GpSimd; h trajectories stored bf16
  4. batched feats^T = W_out @ [h_f; h_b] (+b_out)
  5. Viterbi as two interleaved max-plus scans (forward fv and backward bv)
     in f32; per-step state transpose/replicate via tiny fp32 matmuls
  6. path[t] = argmax_p(fv_t + bv_t - feat_t) batched; score from terminal
"""

import os
import numpy as np
import ml_dtypes

import concourse.bass as bass
import concourse.bacc as bacc
import concourse.mybir as mybir
import concourse.tile as tile
from concourse.bass_utils import run_bass_kernel_spmd

f32 = mybir.dt.float32
bf16 = mybir.dt.bfloat16
i32 = mybir.dt.int32
AF = mybir.ActivationFunctionType
Alu = mybir.AluOpType

N_CORES = 8
B_GLOB, L, E, HID = 32, 512, 256, 512
H = HID // 2
T, START, STOP = 12, 10, 11
NEG = -10000.0
BL = B_GLOB // N_CORES  # 4 seqs per core
NTOK = BL * L           # 2048 tokens per core
NJ = NTOK // 128        # 16 gather columns


def _bf(x):
    return np.asarray(x, dtype=np.float32).astype(ml_dtypes.bfloat16)


def build_kernel():
    nc = bacc.Bacc("TRN2", target_bir_lowering=False, debug=False)

    emb_d = nc.dram_tensor("embt", [50000, E], bf16, kind="ExternalInput")
    idx_d = nc.dram_tensor("idx", [128, NJ], i32, kind="ExternalInput")
    wl_d = nc.dram_tensor("wl", [2, 16, 128, 128], bf16, kind="ExternalInput")
    wi_d = nc.dram_tensor("wi", [2, 16, 128, 128], bf16, kind="ExternalInput")
    bl_d = nc.dram_tensor("bl", [2, 8, 128], bf16, kind="ExternalInput")
    wo_d = nc.dram_tensor("wo", [4, 128, T], bf16, kind="ExternalInput")
    bo_d = nc.dram_tensor("bo", [1, T], bf16, kind="ExternalInput")
    id128_d = nc.dram_tensor("id128", [128, 128], bf16, kind="ExternalInput")
    i12_d = nc.dram_tensor("i12", [T, T], f32, kind="ExternalInput")
    t4f_d = nc.dram_tensor("t4f", [T, BL * T], f32, kind="ExternalInput")
    t4b_d = nc.dram_tensor("t4b", [T, BL * T], f32, kind="ExternalInput")
    fv0_d = nc.dram_tensor("fv0", [T, BL], f32, kind="ExternalInput")
    tstop_d = nc.dram_tensor("tstop", [T, BL], f32, kind="ExternalInput")
    iota_d = nc.dram_tensor("iota", [128, T], f32, kind="ExternalInput")

    score_d = nc.dram_tensor("score", [1, BL], f32, kind="ExternalOutput")
    path_d = nc.dram_tensor("path", [128, NJ], i32, kind="ExternalOutput")

    with tile.TileContext(nc) as tc:
        with (
            tc.tile_pool(name="consts", bufs=1) as consts,
            tc.tile_pool(name="big", bufs=1) as big,
            tc.tile_pool(name="work", bufs=4) as work,
        ):
            # ---------- load constants ----------
            wl_sb = consts.tile([128, 2, 16, 128], bf16, tag="wl")
            nc.sync.dma_start(out=wl_sb, in_=wl_d[:, :, :, :].rearrange("d n p q -> p d n q"))
            wi_sb = consts.tile([128, 2, 16, 128], bf16, tag="wi")
            nc.sync.dma_start(out=wi_sb, in_=wi_d[:, :, :, :].rearrange("d n p q -> p d n q"))
            bl_sb = consts.tile([1, 2, 8, 128], bf16, tag="bl")
            nc.sync.dma_start(out=bl_sb, in_=bl_d[:, :, :].rearrange("d m q -> (d m q)").unsqueeze(0))
            wo_sb = consts.tile([128, 4, T], bf16, tag="wo")
            nc.sync.dma_start(out=wo_sb, in_=wo_d[:, :, :].rearrange("k p t -> p k t"))
            bo_sb = consts.tile([1, T], bf16, tag="bo")
            nc.sync.dma_start(out=bo_sb, in_=bo_d[:, :])
            id128 = consts.tile([128, 128], bf16, tag="id128")
            nc.sync.dma_start(out=id128, in_=id128_d[:, :])
            i12_sb = consts.tile([T, T], f32, tag="i12")
            nc.sync.dma_start(out=i12_sb, in_=i12_d[:, :])
            t4f_sb = consts.tile([T, BL * T], f32, tag="t4f")
            nc.sync.dma_start(out=t4f_sb, in_=t4f_d[:, :])
            t4b_sb = consts.tile([T, BL * T], f32, tag="t4b")
            nc.sync.dma_start(out=t4b_sb, in_=t4b_d[:, :])
            fv0_sb = consts.tile([T, BL], f32, tag="fv0")
            nc.sync.dma_start(out=fv0_sb, in_=fv0_d[:, :])
            tstop_sb = consts.tile([T, BL], f32, tag="tstop")
            nc.sync.dma_start(out=tstop_sb, in_=tstop_d[:, :])
            ones512 = consts.tile([1, 512], bf16, tag="ones512")
            nc.vector.memset(ones512, 1.0)
            iota_sb = consts.tile([128, T], f32, tag="iota")
            nc.sync.dma_start(out=iota_sb, in_=iota_d[:, :])
            ones1f = consts.tile([1, T], f32, tag="ones1f")
            nc.vector.memset(ones1f, 1.0)
            idx_sb = consts.tile([128, NJ], i32, tag="idx")
            nc.sync.dma_start(out=idx_sb, in_=idx_d[:, :])

            # ---------- 1) embedding gather (token u = 4t+s at [u%128, u//128]) ----------
            sc_gather = nc.enter_named_scope("ph_gather", False)
            x_tok = big.tile([128, NJ, E], bf16, tag="xtok")
            for j in range(NJ):
                nc.gpsimd.indirect_dma_start(
                    out=x_tok[:, j, :], out_offset=None,
                    in_=emb_d[:, :],
                    in_offset=bass.IndirectOffsetOnAxis(ap=idx_sb[:, j:j + 1], axis=0),
                )

            # ---------- transpose to x^T [128, 2(kc), NTOK] ----------
            xT = big.tile([128, 2, NTOK], bf16, tag="xT")
            with tc.tile_pool(name="ps_tr", bufs=4, space="PSUM") as ps_tr:
                for j in range(NJ):
                    for kc in range(2):
                        tp = ps_tr.tile([128, 128], f32, tag="tp")
                        nc.tensor.matmul(
                            tp, x_tok[:, j, kc * 128:(kc + 1) * 128], id128,
                            start=True, stop=True)
                        nc.vector.tensor_copy(xT[:, kc, j * 128:(j + 1) * 128], tp)

            nc.leave_named_scope("ph_gather", sc_gather[0], False)
            sc_xw = nc.enter_named_scope("ph_xw", False)
            # ---------- 2) xW precompute ----------
            # xw_buf[d] [128, 8(mc), L, BL] bf16 ; tokens are t-major so an
            # N=512 chunk c covers tokens [512c, 512c+512) = t in [128c,128c+128) x 4 seqs
            xw_buf = big.tile([128, 2, 8, L * BL], bf16, tag="xw")
            with tc.tile_pool(name="ps_xw", bufs=4, space="PSUM") as ps_xw:
                for d in range(2):
                    for mc in range(8):
                        for c in range(NTOK // 512):
                            pxw = ps_xw.tile([128, 512], f32, tag="pxw")
                            for kc in range(2):
                                nc.tensor.matmul(
                                    pxw, wi_sb[:, d, mc * 2 + kc, :],
                                    xT[:, kc, c * 512:(c + 1) * 512],
                                    start=(kc == 0), stop=False)
                            nc.tensor.matmul(
                                pxw, bl_sb[:, d, mc, :],
                                ones512,
                                start=False, stop=True)
                            nc.vector.tensor_copy(
                                xw_buf[:, d, mc, c * 512:(c + 1) * 512], pxw)

            nc.leave_named_scope("ph_xw", sc_xw[0], False)
            sc_lstm = nc.enter_named_scope("ph_lstm", False)
            # ---------- 3) LSTM fused pass ----------
            hT_buf = big.tile([128, 2, 2, L, BL], bf16, tag="hT")  # [p, d, kc, t, s]
            h_init = consts.tile([128, 2, BL], bf16, tag="hinit")
            nc.vector.memset(h_init.rearrange("p k b -> p (k b)"), 0.0)
            c_init = []
            for d in range(2):
                ci = consts.tile([128, 2 * BL], f32, tag=f"cinit{d}")
                nc.vector.memset(ci, 0.0)
                c_init.append(ci)
            c_cur = list(c_init)

            with (
                tc.tile_pool(name="ps_g", bufs=2, space="PSUM") as ps_g,
                tc.tile_pool(name="lwork", bufs=4) as lwork,
                tc.tile_pool(name="lstate", bufs=3) as lstate,
            ):
                for step in range(L):
                    for d in range(2):
                        tp = step if d == 0 else L - 1 - step
                        tprev = tp - 1 if d == 0 else tp + 1
                        if step == 0:
                            h_prev = h_init[:, :, :]
                        else:
                            h_prev = hT_buf[:, d, :, tprev, :]
                        c_prev = c_cur[d]
                        g_ps = ps_g.tile([128, 8, BL], f32, tag=f"g{d}")
                        for mc in range(8):
                            for kc in range(2):
                                nc.tensor.matmul(
                                    g_ps[:, mc, :],
                                    wl_sb[:, d, mc * 2 + kc, :],
                                    h_prev[:, kc, :],
                                    start=(kc == 0), stop=(kc == 1))
                        g_sb = lwork.tile([128, 8 * BL], f32, tag=f"gsb{d}")
                        nc.vector.tensor_add(
                            g_sb.rearrange("p (m b) -> p m b", m=8),
                            g_ps,
                            xw_buf[:, d, :, tp * BL:(tp + 1) * BL])
                        sg = lwork.tile([128, 8 * BL], f32, tag=f"sg{d}")
                        nc.scalar.activation(sg, g_sb, AF.Sigmoid)
                        tg = lwork.tile([128, 2 * BL], f32, tag=f"tg{d}")
                        nc.scalar.activation(tg, g_sb[:, 4 * BL:6 * BL], AF.Tanh)
                        p_t = lwork.tile([128, 2 * BL], f32, tag=f"p{d}")
                        nc.gpsimd.tensor_mul(p_t, sg[:, 0:2 * BL], tg)
                        m_t = lwork.tile([128, 2 * BL], f32, tag=f"m{d}")
                        nc.vector.tensor_mul(m_t, sg[:, 2 * BL:4 * BL], c_prev)
                        c2 = lstate.tile([128, 2 * BL], f32, tag=f"c{d}")
                        nc.vector.tensor_add(c2, p_t, m_t)
                        th = lwork.tile([128, 2 * BL], f32, tag=f"th{d}")
                        nc.scalar.activation(th, c2, AF.Tanh)
                        nc.gpsimd.tensor_mul(
                            hT_buf[:, d, :, tp, :],
                            sg[:, 6 * BL:8 * BL].rearrange("p (k b) -> p k b", k=2),
                            th.rearrange("p (k b) -> p k b", k=2))
                        c_cur[d] = c2

            nc.leave_named_scope("ph_lstm", sc_lstm[0], False)
            sc_feat = nc.enter_named_scope("ph_feats", False)
            # ---------- 4) feats^T [T, L, BL] ----------
            featsT = big.tile([T, L, BL], f32, tag="featsT")
            with tc.tile_pool(name="ps_f", bufs=4, space="PSUM") as ps_f:
                for c in range(NTOK // 512):
                    pf = ps_f.tile([T, 512], f32, tag="pf")
                    for d in range(2):
                        for kc in range(2):
                            nc.tensor.matmul(
                                pf, wo_sb[:, d * 2 + kc, :],
                                hT_buf[:, d, kc, c * 128:(c + 1) * 128, :].rearrange("p t b -> p (t b)"),
                                start=(d == 0 and kc == 0), stop=False)
                    nc.tensor.matmul(
                        pf, bo_sb[:, :], ones512,
                        start=False, stop=True)
                    nc.vector.tensor_copy(
                        featsT[:, c * 128:(c + 1) * 128, :].rearrange("p t b -> p (t b)"), pf)

            nc.leave_named_scope("ph_feats", sc_feat[0], False)
            sc_vit = nc.enter_named_scope("ph_viterbi", False)
            # ---------- 5) Viterbi double scan ----------
            fv_buf = big.tile([T, L, BL], f32, tag="fv")
            bv_buf = big.tile([T, L, BL], f32, tag="bv")

            with (
                tc.tile_pool(name="ps_v", bufs=4, space="PSUM") as ps_v,
                tc.tile_pool(name="ps_p", bufs=2, space="PSUM") as ps_p,
                tc.tile_pool(name="vwork", bufs=4) as vwork,
            ):
                # bwd init: bv[511] = tstop + feat[511]
                nc.vector.tensor_add(bv_buf[:, L - 1, :], tstop_sb, featsT[:, L - 1, :])

                def scan_step(t4_sb, state_prev_col, buf, t_out):
                    # sc = T4 + replicate(state_prev) via PE accumulation
                    sc_ps = ps_v.tile([T, BL, T], f32, tag="sc")
                    nc.tensor.matmul(
                        sc_ps.rearrange("p s j -> p (s j)"), i12_sb, t4_sb,
                        start=True, stop=False)
                    for s in range(BL):
                        nc.tensor.matmul(
                            sc_ps[:, s, :],
                            state_prev_col[:, s:s + 1].broadcast_to([T, T]),
                            i12_sb, start=False, stop=(s == BL - 1))
                    mx = vwork.tile([T, BL], f32, tag="mx")
                    nc.vector.tensor_reduce(
                        mx.unsqueeze(-1), sc_ps,
                        axis=mybir.AxisListType.X, op=Alu.max)
                    nc.vector.tensor_add(buf[:, t_out, :], mx, featsT[:, t_out, :])

                for step in range(L):
                    # forward chain at t = step
                    prev_f = fv0_sb if step == 0 else fv_buf[:, step - 1, :]
                    scan_step(t4f_sb, prev_f, fv_buf, step)
                    # backward chain at t = L-2-step
                    t_b = L - 2 - step
                    if t_b >= 0:
                        scan_step(t4b_sb, bv_buf[:, t_b + 1, :], bv_buf, t_b)

                nc.leave_named_scope("ph_viterbi", sc_vit[0], False)
                sc_path = nc.enter_named_scope("ph_path", False)
                # ---------- 6) score + path ----------
                term = vwork.tile([T, BL], f32, tag="term")
                nc.vector.tensor_add(term, fv_buf[:, L - 1, :], tstop_sb)
                trs = ps_p.tile([1, BL * T], f32, tag="tr")
                for s in range(BL):
                    nc.tensor.matmul(trs[:, s * T:(s + 1) * T],
                                     term[:, s:s + 1], i12_sb, start=True, stop=True)
                termT = vwork.tile([1, BL * T], f32, tag="termT")
                nc.vector.tensor_copy(termT, trs)
                score_sb = vwork.tile([1, BL], f32, tag="score")
                nc.vector.tensor_reduce(
                    score_sb.unsqueeze(-1),
                    termT.rearrange("p (s j) -> p s j", s=BL),
                    axis=mybir.AxisListType.X, op=Alu.max)
                nc.sync.dma_start(out=score_d[:, :], in_=score_sb)

                # tot = fv + bv - feats  [T, L*BL]
                tot0 = big.tile([T, L, BL], f32, tag="tot0")
                nc.vector.tensor_add(
                    tot0.rearrange("p t b -> p (t b)"),
                    fv_buf.rearrange("p t b -> p (t b)"),
                    bv_buf.rearrange("p t b -> p (t b)"))
                tot = big.tile([T, L, BL], f32, tag="tot")
                nc.vector.tensor_sub(
                    tot.rearrange("p t b -> p (t b)"),
                    tot0.rearrange("p t b -> p (t b)"),
                    featsT.rearrange("p t b -> p (t b)"))
                totT = big.tile([128, NJ, T], f32, tag="totT")
                for j in range(NJ):
                    ptt = ps_p.tile([128, T], f32, tag="tr")
                    nc.tensor.matmul(
                        ptt,
                        tot.rearrange("p t b -> p (t b)")[:, j * 128:(j + 1) * 128],
                        i12_sb, start=True, stop=True)
                    nc.vector.tensor_copy(totT[:, j, :], ptt)
                totmax = vwork.tile([128, NJ], f32, tag="totmax")
                nc.vector.tensor_reduce(
                    totmax.unsqueeze(-1), totT,
                    axis=mybir.AxisListType.X, op=Alu.max)
                ohp = big.tile([128, NJ, T], f32, tag="ohp")
                nc.vector.tensor_tensor(
                    out=ohp, in0=totT,
                    in1=totmax.unsqueeze(-1).broadcast_to([128, NJ, T]),
                    op=Alu.is_equal)
                pathv = vwork.tile([128, NJ], f32, tag="pathv")
                wsum = big.tile([128, NJ, T], f32, tag="wsum")
                nc.vector.tensor_mul(
                    wsum, ohp,
                    iota_sb.unsqueeze(1).broadcast_to([128, NJ, T]))
                nc.vector.tensor_reduce(
                    pathv.unsqueeze(-1), wsum,
                    axis=mybir.AxisListType.X, op=Alu.add)
                path_i = vwork.tile([128, NJ], i32, tag="pathi")
                nc.vector.tensor_copy(path_i, pathv)
                nc.sync.dma_start(out=path_d[:, :], in_=path_i)
                nc.leave_named_scope("ph_path", sc_path[0], False)

    nc.compile()
    return nc


def _host_prep(sentence, emb, W_ih_f, W_hh_f, b_f, W_ih_b, W_hh_b, b_b,
               W_out, b_out, transitions):
    """Build per-core input maps."""
    sent = np.asarray(sentence)
    embt = _bf(emb)
    trans = np.asarray(transitions, np.float32)

    def pack_w(W):  # W [4H=1024, 256] -> [16, 128, 128] tiles lhsT[kc-chunk, mc-chunk]
        Wt = np.asarray(W, np.float32).T  # [256, 1024]
        tiles = np.zeros((16, 128, 128), np.float32)
        for mc in range(8):
            for kc in range(2):
                tiles[mc * 2 + kc] = Wt[kc * 128:(kc + 1) * 128, mc * 128:(mc + 1) * 128]
        return _bf(tiles)

    wl = np.stack([pack_w(W_hh_f), pack_w(W_hh_b)])       # [2,16,128,128]
    wi = np.stack([pack_w(W_ih_f), pack_w(W_ih_b)])
    bl = np.stack([_bf(np.asarray(b_f, np.float32).reshape(8, 128)),
                   _bf(np.asarray(b_b, np.float32).reshape(8, 128))])
    WoT = np.asarray(W_out, np.float32).T                 # [512, 12]
    wo = _bf(np.stack([WoT[0:128], WoT[128:256], WoT[256:384], WoT[384:512]]))
    bo = _bf(np.asarray(b_out, np.float32).reshape(1, T))
    id128 = _bf(np.eye(128, dtype=np.float32))
    i12 = np.eye(T, dtype=np.float32)
    t4f = np.tile(trans[:, None, :], (1, BL, 1)).reshape(T, BL * T)
    t4b = np.tile(trans.T[:, None, :], (1, BL, 1)).reshape(T, BL * T)
    fv0 = np.full((T, BL), NEG, np.float32)
    fv0[START, :] = 0.0
    tstop = np.tile(trans[STOP][:, None], (1, BL)).astype(np.float32)
    iota = np.tile(np.arange(T, dtype=np.float32)[None, :], (128, 1))

    in_maps = []
    for c in range(N_CORES):
        seqs = sent[c * BL:(c + 1) * BL]  # [4, 512]
        idx = np.zeros((128, NJ), np.int32)
        for u in range(NTOK):
            t, s = divmod(u, BL)
            idx[u % 128, u // 128] = seqs[s, t]
        in_maps.append(dict(
            embt=embt, idx=idx, wl=wl, wi=wi, bl=bl, wo=wo, bo=bo,
            id128=id128, i12=i12, t4f=t4f, t4b=t4b, fv0=fv0, tstop=tstop,
            iota=iota,
        ))
    return in_maps


def kernel(sentence, emb, W_ih_f, W_hh_f, b_f, W_ih_b, W_hh_b, b_b,
           W_out, b_out, transitions):
    in_maps = _host_prep(sentence, emb, W_ih_f, W_hh_f, b_f,
                         W_ih_b, W_hh_b, b_b, W_out, b_out, transitions)
    nc = build_kernel()
    trace = os.environ.get("BILSTM_TRACE", "0") == "1"
    res = run_bass_kernel_spmd(nc, in_maps, core_ids=list(range(N_CORES)),
                               trace=trace)
    if trace:
        print(f"HW exec time: {res.exec_time_ns} ns")
        for k, v in sorted((res.per_core_scope_times or {}).items()):
            print(f"  scope {k}: {list(v.values())[0]} ns")
    score = np.zeros((B_GLOB,), np.float32)
    path = np.zeros((B_GLOB, L), np.int32)
    for c in range(N_CORES):
        r = res.results[c]
        score[c * BL:(c + 1) * BL] = r["score"][0]
        pt = r["path"]  # [128, NJ] token-major u = 4t+s
        for u in range(NTOK):
            t, s = divmod(u, BL)
            path[c * BL + s, t] = pt[u % 128, u // 128]
    return score, path
